# revision 1
# baseline (speedup 1.0000x reference)
"""Trainium2 Bass kernel for histogram_binning (windowed-cosine binning).

Reference computation (per element):
    d = x[k,i] - phis[i,j]
    out[k, i*L+j] = 0.5*cos(d)+0.5  if  -interval[i] < d <= interval[i]  else 0

Strategy (8 cores, data-parallel over batch):
  - Each core handles a 128-row batch shard; output shard is [128, 65536] f32
    (32MB) -> the kernel is HBM-write bound (~358 GB/s/core => ~94us floor).
  - On-chip layout: partition dim = feature i (two halves of 128), free dim =
    (k_block, j).  phis half [128,256] and interval half [128,1] stay resident;
    x arrives host-transposed as [256,128] so x[:,k] columns are per-partition
    scalars.
  - Two compute routes per chunk (mixable, to balance DVE vs ACT):
    "bigfd": per batch row k one dual-op tensor_scalar computes d=(phis*-1)+x
      exactly (DVE 2x fp32); ACT computes c=sin(d+pi/2) on the whole chunk
      (large free dim amortizes the 224cyc ACT overhead); one fused custom DVE
      op applies window+value: ((d<=iv)&(d+iv>0)) * ((c+1)*0.5).
    "perk": fully-fused custom DVE op per row recomputes d=x-phi in-op
      (8 ALU stages) so DVE skips the d pass; ACT computes c per row from
      phis directly (sin(-phi + (x+pi/2))).
  - Window compares use the exactly-rounded fp32 d, matching the reference's
    float semantics bit-for-bit ((d+iv>0) <=> (d>-iv) holds exactly in fp32;
    subtraction/compare are exact IEEE ops on both engines).
"""

import math
import os

import numpy as np

import concourse.bacc as bacc
import concourse.mybir as mybir
from concourse import dve_ops
from concourse.bass_utils import run_bass_kernel_spmd
from concourse.dve_spec import (
    C0,
    C1,
    C2,
    One,
    Spec,
    Src0,
    Src1,
    Zero,
    _has_src1,
    lower,
)
from concourse.dve_uop import DveOpSpec
from concourse.tile import TileContext

B, M, L = 1024, 256, 256
N_CORES = 8
B_SHARD = B // N_CORES  # 128
HALF = 128  # features per partition-half
F32 = mybir.dt.float32
HALF_PI = float(np.pi / 2)

_OPS_CACHE = {}


def _register_op(name, spec):
    """Register a custom DVE op under `name`, computing its uops sha."""
    if name in _OPS_CACHE:
        return _OPS_CACHE[name]
    for existing in dve_ops.OPS:
        if existing.name == name:
            _OPS_CACHE[name] = existing
            return existing
    if name not in dve_ops._SUB_OPCODE_FOR_NAME:
        row = max(dve_ops._SUB_OPCODE_FOR_NAME.values()) + 1
        assert row < 0x20, "no free custom-DVE opcode rows"
        dve_ops._SUB_OPCODE_FOR_NAME[name] = row
    shas = {}
    for ver in ("v3", "v4"):
        uops = lower(spec, ver=ver)
        shas[ver] = DveOpSpec(
            name=name,
            opcode=dve_ops.get_dve_sub_opcode(name),
            uops=uops,
            rd1_en=_has_src1(spec),
        ).sha(ver)
    op = dve_ops.DveOp(name, spec, subdim=False, uops_sha=shas)
    dve_ops.OPS.append(op)
    dve_ops.CUSTOM_DVE_SPECS[name] = spec
    _OPS_CACHE[name] = op
    return op


def _get_winsel_op():
    """out = ((d <= iv) & (d > -iv)) * (0.5*c + 0.5)
    Src0 = d, Src1 = c, C0 = iv [P,1], C2 = 0.5.  -iv is a hoisted
    stream-invariant const (zero body stages).  6 ALU stages."""
    cond = (Src0 <= C0) & (Src0 > (Zero - C0))
    body = cond * (Src1 * C2 + C2)

    def _ref(in0, in1, s0, s1, imm2):
        f = np.float32
        m = (in0 <= s0) & (in0 > (f(0.0) - s0))
        return (m.astype(np.float32) * (in1 * f(imm2) + f(imm2))).astype(np.float32)

    return _register_op("WINSEL_COS_ANT", Spec(body=body, reference=_ref))


def _get_winsel_full_op():
    """Fully fused: d = x - phi computed in-op.
    out = ((d <= iv) & (d > -iv)) * (0.5*c + 0.5)
    Src0 = phi, Src1 = c, C0 = iv [P,1], C1 = x [P,1], C2 = 0.5.  7 stages."""
    d = C1 - Src0
    cond = (d <= C0) & (d > (Zero - C0))
    body = cond * (Src1 * C2 + C2)

    def _ref(in0, in1, s0, s1, imm2):
        f = np.float32
        dd = (s1 - in0).astype(np.float32)
        m = (dd <= s0) & (dd > (f(0.0) - s0))
        return (m.astype(np.float32) * (in1 * f(imm2) + f(imm2))).astype(np.float32)

    return _register_op("WINSEL_COS_FULL_ANT", Spec(body=body, reference=_ref))


def build_nc(
    variant="v2",
    K=16,
    perk_frac=0.0,
    num_devices=N_CORES,
    bufs=None,
    reps=1,
    gfrac=0.0,
):
    """Build the per-core Bass program.

    variant "v2": custom DVE ops (fast path); perk_frac in [0,1] selects the
      fraction of chunks routed through the fully-fused per-row op (shifts
      load from DVE to ACT).
    variant "v1": no custom ops -- ACT sin + explicit mask arithmetic.
    K: batch rows per chunk (free-dim tile = K*256 f32 per chunk).
    """
    assert B_SHARD % K == 0
    n_chunks = B_SHARD // K
    mult, add = mybir.AluOpType.mult, mybir.AluOpType.add
    is_le, is_gt = mybir.AluOpType.is_le, mybir.AluOpType.is_gt

    nc = bacc.Bacc(
        "TRN2",
        target_bir_lowering=False,
        debug=False,
        enable_asserts=True,
        num_devices=num_devices,
    )
    xt_d = nc.dram_tensor("xt", [M, B_SHARD], F32, kind="ExternalInput")
    ph_d = nc.dram_tensor("phis", [M, L], F32, kind="ExternalInput")
    iv_d = nc.dram_tensor("interval", [M], F32, kind="ExternalInput")
    y_d = nc.dram_tensor("out", [B_SHARD, M * L], F32, kind="ExternalOutput")
    # out[k, (h*128+i)*256 + j] viewed as [h, i(part), k, j]
    yr = y_d.ap().rearrange("b (h i j) -> h i b j", h=2, i=HALF, j=L)
    ivr = iv_d.ap().rearrange("(h i one) -> h i one", h=2, one=1)
    xtr = xt_d.ap().rearrange("(h i) b -> h i b", h=2)
    phr = ph_d.ap().rearrange("(h i) j -> h i j", h=2)

    if variant == "v2":
        winsel = _get_winsel_op()
        winsel_full = _get_winsel_full_op() if perk_frac > 0 else None
    # Bresenham spread of per-k chunks among the n_chunks of each half
    n_perk = int(round(perk_frac * n_chunks))
    route_perk = [
        (ci + 1) * n_perk // n_chunks > ci * n_perk // n_chunks
        for ci in range(n_chunks)
    ]
    # Bresenham spread of d-pass rows onto GPSIMD (third compute engine)
    n_rows = 2 * B_SHARD
    n_g = int(round(gfrac * n_rows))
    route_g = [
        (r + 1) * n_g // n_rows > r * n_g // n_rows for r in range(n_rows)
    ]

    if bufs is None:
        bufs = 5 if K <= 8 else (3 if K <= 16 else 2)
    with TileContext(nc) as tc:
        with (
            tc.tile_pool(name="const", bufs=1) as cpool,
            tc.tile_pool(name="dwork", bufs=bufs) as dpool,
            tc.tile_pool(name="cwork", bufs=bufs) as cwpool,
            tc.tile_pool(name="owork", bufs=bufs) as opool,
            tc.tile_pool(name="scratch", bufs=2) as spool,
        ):
            hp_t = cpool.tile([HALF, 1], F32, tag="halfpi")
            nc.gpsimd.memset(hp_t[:], HALF_PI)
            # Trigger the Sin table-set load (~2.7us) while input DMAs fly.
            warm_t = cpool.tile([HALF, 1], F32, tag="warm")
            nc.scalar.activation(
                warm_t[:], hp_t[:], mybir.ActivationFunctionType.Sin,
                bias=0.0, scale=0.0,
            )
            ph_t, iv_t, xt_t, niv_t, xb_t = [], [], [], [], []
            for h in range(2):
                p = cpool.tile([HALF, L], F32, tag=f"ph{h}")
                nc.sync.dma_start(out=p[:], in_=phr[h])
                ph_t.append(p)
                i_ = cpool.tile([HALF, 1], F32, tag=f"iv{h}")
                nc.sync.dma_start(out=i_[:], in_=ivr[h])
                iv_t.append(i_)
                xt = cpool.tile([HALF, B_SHARD], F32, tag=f"xt{h}")
                nc.sync.dma_start(out=xt[:], in_=xtr[h])
                xt_t.append(xt)
                if variant == "v1":
                    ni = cpool.tile([HALF, 1], F32, tag=f"niv{h}")
                    nc.vector.tensor_scalar(
                        out=ni[:], in0=i_[:], scalar1=-1.0, scalar2=None, op0=mult
                    )
                    niv_t.append(ni)
                if n_perk > 0:
                    xb = cpool.tile([HALF, B_SHARD], F32, tag=f"xb{h}")
                    # xb = x + pi/2 (ACT bias for the per-k route)
                    nc.vector.tensor_scalar(
                        out=xb[:], in0=xt[:], scalar1=HALF_PI, scalar2=None, op0=add
                    )
                    xb_t.append(xb)

            def emit_chunk(h, ci):
                    o = opool.tile([HALF, K * L], F32, tag="o")
                    if variant == "v2" and route_perk[ci]:
                        # fully-fused route: ACT computes c from phis per row;
                        # custom op recomputes d in-op.
                        c = cwpool.tile([HALF, K * L], F32, tag="c")
                        for k in range(K):
                            kg = ci * K + k
                            sl = slice(k * L, (k + 1) * L)
                            nc.scalar.activation(
                                c[:, sl],
                                ph_t[h][:],
                                mybir.ActivationFunctionType.Sin,
                                bias=xb_t[h][:, kg : kg + 1],
                                scale=-1.0,
                            )
                            nc.vector._custom_dve(
                                winsel_full,
                                out=o[:, sl],
                                in0=ph_t[h][:],
                                in1=c[:, sl],
                                s0=iv_t[h][:],
                                s1=xt_t[h][:, kg : kg + 1],
                                imm2=0.5,
                            )
                    else:
                        d = dpool.tile([HALF, K * L], F32, tag="d")
                        for k in range(K):
                            kg = ci * K + k
                            eng = (
                                nc.gpsimd
                                if route_g[h * B_SHARD + kg]
                                else nc.vector
                            )
                            eng.tensor_scalar(
                                out=d[:, k * L : (k + 1) * L],
                                in0=ph_t[h][:],
                                scalar1=-1.0,
                                scalar2=xt_t[h][:, kg : kg + 1],
                                op0=mult,
                                op1=add,
                            )
                        c = cwpool.tile([HALF, K * L], F32, tag="c")
                        nc.scalar.activation(
                            c[:],
                            d[:],
                            mybir.ActivationFunctionType.Sin,
                            bias=hp_t[:],
                            scale=1.0,
                        )
                        if variant == "v2":
                            nc.vector._custom_dve(
                                winsel,
                                out=o[:],
                                in0=d[:],
                                in1=c[:],
                                s0=iv_t[h][:],
                                s1=0.0,
                                imm2=0.5,
                            )
                        else:  # v1
                            a = spool.tile([HALF, K * L], F32, tag="a")
                            # a = (d <= iv) - 1  in {-1, 0}
                            nc.vector.tensor_scalar(
                                out=a[:], in0=d[:], scalar1=iv_t[h][:],
                                scalar2=-1.0, op0=is_le, op1=add,
                            )
                            b = spool.tile([HALF, K * L], F32, tag="b")
                            # b = (d > -iv)  in {0, 1}
                            nc.vector.tensor_scalar(
                                out=b[:], in0=d[:], scalar1=niv_t[h][:],
                                scalar2=None, op0=is_gt,
                            )
                            m = spool.tile([HALF, K * L], F32, tag="m")
                            nc.vector.tensor_tensor(
                                out=m[:], in0=a[:], in1=b[:], op=add
                            )
                            v = spool.tile([HALF, K * L], F32, tag="v")
                            # v = (c + 1) * 0.5
                            nc.vector.tensor_scalar(
                                out=v[:], in0=c[:], scalar1=1.0,
                                scalar2=0.5, op0=add, op1=mult,
                            )
                            nc.vector.tensor_tensor(
                                out=o[:], in0=m[:], in1=v[:], op=mult
                            )
                    nc.sync.dma_start(out=yr[h, :, ci * K : (ci + 1) * K, :], in_=o[:])

            import contextlib

            loop_ctx = (
                tc.For_i(0, reps, 1, hint_engines=tuple(mybir.ALL_ENGINES))
                if reps > 1
                else contextlib.nullcontext()
            )
            with loop_ctx:
                for h in range(2):
                    for ci in range(n_chunks):
                        emit_chunk(h, ci)
    nc.compile()
    return nc


_NC_CACHE = {}


def _build_cfg():
    variant = os.environ.get("HB_VARIANT", "v2")
    K = int(os.environ.get("HB_K", "8"))
    perk = float(os.environ.get("HB_PERK", "0.75" if variant == "v2" else "0.0"))
    gfrac = float(os.environ.get("HB_GFRAC", "0.0"))
    return variant, K, perk, gfrac


def _get_nc():
    key = _build_cfg()
    if key not in _NC_CACHE:
        variant, K, perk, gfrac = key
        _NC_CACHE[key] = build_nc(
            variant=variant, K=K, perk_frac=perk, gfrac=gfrac
        )
    return _NC_CACHE[key]


def kernel(x, phis, interval):
    x = np.ascontiguousarray(x, dtype=np.float32)
    phis = np.ascontiguousarray(phis, dtype=np.float32)
    interval = np.ascontiguousarray(interval, dtype=np.float32)
    assert x.shape == (B, M) and phis.shape == (M, L) and interval.shape == (M,)

    nc = _get_nc()
    in_maps = []
    for c in range(N_CORES):
        shard = x[c * B_SHARD : (c + 1) * B_SHARD]
        in_maps.append(
            {
                "xt": np.ascontiguousarray(shard.T),
                "phis": phis,
                "interval": interval,
            }
        )
    res = run_bass_kernel_spmd(nc, in_maps, core_ids=list(range(N_CORES)))
    return np.concatenate(
        [res.results[c]["out"] for c in range(N_CORES)], axis=0
    )

